# revision 17
# baseline (speedup 1.0000x reference)
"""Trainium2 Bass kernel for nn_Encoder (3-stage neighborhood-attention encoder).

Sharding: 8 cores = 4 batches x 2 token-halves. Each core:
  - projects K/V for its own half of tokens, AllGathers the KV table with its
    twin core (same batch, other half),
  - runs Q-projection, neighbor gather (dma_gather), softmax-attention,
    output projection and patch-merge for its own half only.
Between stages the patch-merge output feeds the next stage entirely locally
(merging is row-local), so the KV AllGather is the only cross-core traffic.

kernel(**inputs) takes the FULL inputs from reference.setup_inputs() and
returns the FULL output tuple (x_final, skip0, skip1, skip2).
"""

import os
import numpy as np

os.environ.setdefault("MYCRO_LOCAL_CACHE", "1")

import ml_dtypes  # noqa: E402
from concourse import bass, bacc, tile, mybir  # noqa: E402
from concourse.bass_utils import run_bass_kernel_spmd  # noqa: E402
from concourse.library_config import mlp as MLP_LIB  # noqa: E402

AF = mybir.ActivationFunctionType
ALU = mybir.AluOpType
F32 = mybir.dt.float32
BF16 = mybir.dt.bfloat16
AX = mybir.AxisListType

P = 128
KN = 8  # neighbors per query
EPS = 1e-5


# ---------------------------------------------------------------- config ----
class Cfg:
    def __init__(self, H0=128, W0=128, C0=96, act_dt=BF16, n_pairs=4, split=2):
        self.H0, self.W0, self.C0 = H0, W0, C0
        self.act_dt = act_dt
        self.n_pairs = n_pairs          # batch elements
        self.split = split              # token-halves per batch (2 = twin cores)
        self.collective = split == 2    # twin-core KV AllGather
        self.stages = []
        for s in range(3):
            C = C0 * (1 << s)
            H = H0 >> s
            W = W0 >> s
            N = H * W
            Nh = N // split
            nt = Nh // P                    # query tiles per core
            assert Nh % P == 0
            G = max(1, min(4 >> s, nt))      # tiles per gather block
            assert nt % G == 0
            # gather row length in elements (pad to 256B multiples)
            esz = mybir.dt.size(act_dt)
            R = 2 * C
            while (R * esz) % 256:
                R += 1
            self.stages.append(dict(C=C, H=H, W=W, N=N, Nh=Nh, nt=nt, G=G, R=R))

    def key(self):
        return (self.H0, self.W0, self.C0, str(self.act_dt), self.n_pairs,
                self.split)


def _chunks(C):
    """Split feature dim C into partition-sized chunks."""
    out = []
    c0 = 0
    while c0 < C:
        c1 = min(c0 + P, C)
        out.append((c0, c1))
        c0 = c1
    return out


# ------------------------------------------------------------- builder ------
def build_encoder(cfg: Cfg):
    ACT = cfg.act_dt
    nc = bacc.Bacc("TRN2", target_bir_lowering=False, debug=False,
                   num_devices=cfg.split * cfg.n_pairs)

    # ---- external tensors ----
    s0 = cfg.stages[0]
    x_in = nc.dram_tensor("x", [s0["H"] // cfg.split, s0["W"], s0["C"]], F32,
                          kind="ExternalInput").ap()
    id_in = nc.dram_tensor("ident", [P, P], ACT, kind="ExternalInput").ap()
    idx_in, w_in, outs = [], [], []
    for s, st in enumerate(cfg.stages):
        C, Nh = st["C"], st["Nh"]
        idx_in.append(nc.dram_tensor(f"idx{s}", [P, Nh * KN // 16], mybir.dt.int16,
                                     kind="ExternalInput").ap())
        w_in.append(dict(
            wqkv=nc.dram_tensor(f"wqkv{s}", [C, 3 * C], ACT, kind="ExternalInput").ap(),
            bqkv=nc.dram_tensor(f"bqkv{s}", [1, 3 * C], ACT, kind="ExternalInput").ap(),
            wo=nc.dram_tensor(f"wo{s}", [C, C], ACT, kind="ExternalInput").ap(),
            bo=nc.dram_tensor(f"bo{s}", [1, C], ACT, kind="ExternalInput").ap(),
            wrp=nc.dram_tensor(f"wrp{s}", [4 * C, 2 * C], ACT, kind="ExternalInput").ap(),
            brp=nc.dram_tensor(f"brp{s}", [1, 2 * C], ACT, kind="ExternalInput").ap(),
            b128=nc.dram_tensor(f"b128_{s}", [P, KN], F32, kind="ExternalInput").ap(),
        ))
        outs.append(nc.dram_tensor(f"skip{s}", [Nh, C], F32, kind="ExternalOutput").ap())
    st2 = cfg.stages[2]
    n_xout = st2["Nh"] // 4
    xout = nc.dram_tensor("xout2", [n_xout, 2 * st2["C"]], F32,
                          kind="ExternalOutput").ap()

    # ---- internal DRAM ----
    kvh, kvf, skip_scr = [], [], []
    for s, st in enumerate(cfg.stages):
        kvh.append(nc.dram_tensor(f"kvh{s}", [st["Nh"], st["R"]], ACT,
                                  kind="Internal").ap())
        if cfg.collective:
            kvf.append(nc.dram_tensor(f"kvf{s}", [st["N"], st["R"]], ACT,
                                      kind="Internal").ap())
        else:
            kvf.append(kvh[s])
        skip_scr.append(nc.dram_tensor(f"sscr{s}", [st["Nh"], st["C"]], F32,
                                       kind="Internal").ap())

    with tile.TileContext(nc) as tc:
        # ---- constants / weights in SBUF (persistent bufs=1 pool) ----
        wp_cm = tc.tile_pool(name="wp", bufs=1)
        wp = wp_cm.__enter__()

        def single(shape, dtype, nm):
            return wp.tile(shape, dtype, tag=nm, name=nm)

        ident = single([P, P], ACT, "identsb")
        nc.sync.dma_start(out=ident[:], in_=id_in)
        ll = (nc.gpsimd.load_library(MLP_LIB) if os.environ.get("KM_LOADLIB")
              else nc.gpsimd.engine_nop())
        ones = single([1, 512], ACT, "ones")
        nc.vector.memset(ones[:], 1.0)
        eps1 = single([P, 1], F32, "eps1")
        nc.vector.memset(eps1[:], float(EPS))
        zero1 = single([P, 1], F32, "zero1")
        nc.vector.memset(zero1[:], 0.0)

        wsb = []
        for s, st in enumerate(cfg.stages):
            C = st["C"]
            d = {}
            d["wqkv"] = []
            d["wo"] = []
            d["wrp"] = []
            for i, (c0, c1) in enumerate(_chunks(C)):
                t = single([c1 - c0, 3 * C], ACT, f"wqkv{s}_{i}")
                nc.sync.dma_start(out=t[:], in_=w_in[s]["wqkv"][c0:c1, :])
                d["wqkv"].append(t)
                t = single([c1 - c0, C], ACT, f"wo{s}_{i}")
                nc.sync.dma_start(out=t[:], in_=w_in[s]["wo"][c0:c1, :])
                d["wo"].append(t)
            for i, (c0, c1) in enumerate(_chunks(4 * C)):
                t = single([c1 - c0, 2 * C], ACT, f"wrp{s}_{i}")
                nc.sync.dma_start(out=t[:], in_=w_in[s]["wrp"][c0:c1, :])
                d["wrp"].append(t)
            for nm, sh in (("bqkv", [1, 3 * C]), ("bo", [1, C]), ("brp", [1, 2 * C])):
                t = single(sh, ACT, f"{nm}{s}")
                nc.sync.dma_start(out=t[:], in_=w_in[s][nm])
                d[nm] = t
            t = single([P, KN], F32, f"b128s{s}")
            nc.sync.dma_start(out=t[:], in_=w_in[s]["b128"])
            d["b128"] = t
            d["idx"] = single([P, st["Nh"] * KN // 16], mybir.dt.int16,
                              f"idxsb{s}")
            nc.sync.dma_start(out=d["idx"][:], in_=idx_in[s])
            wsb.append(d)

        # persistent activation tiles (xT chunks per stage)
        xt = []
        for s, st in enumerate(cfg.stages):
            xt.append([single([c1 - c0, st["Nh"]], ACT, f"xt{s}_{i}")
                       for i, (c0, c1) in enumerate(_chunks(st["C"]))])

        # pools
        sb_cm = tc.tile_pool(name="sb", bufs=3)
        sb = sb_cm.__enter__()
        gp_cm = tc.tile_pool(name="gp", bufs=2)
        gp = gp_cm.__enter__()
        ps_cm = tc.tile_pool(name="ps", bufs=4, space="PSUM")
        ps = ps_cm.__enter__()
        ps_tr_cm = tc.tile_pool(name="ps_tr", bufs=3, space="PSUM")
        ps_tr = ps_tr_cm.__enter__()

        def mm(out_ap, lhsT_ap, rhs_ap, start, stop):
            nc.tensor.matmul(out_ap, lhsT_ap, rhs_ap, start=start, stop=stop)

        # ============================ stages ============================
        _nstages = int(os.environ.get("KM_NSTAGES", "3"))
        _phases = os.environ.get("KM_PHASES", "ABCD")
        for s, st in enumerate(cfg.stages):
            if s >= _nstages:
                break
            C, Nh, nt, G, R = st["C"], st["Nh"], st["nt"], st["G"], st["R"]
            W = st["W"]
            CH = _chunks(C)
            nblocks = nt // G
            rsqc = 1.0  # 1/sqrt(C) folded into wq on host

            # ---- stage 0: load x and build xT (feature-major) ----
            if s == 0:
                xflat = x_in.rearrange("h w c -> (h w) c")
                for t in range(nt):
                    x_tm = sb.tile([P, C], ACT, tag="x_tm")
                    if ACT == F32:
                        nc.sync.dma_start(out=x_tm[:], in_=xflat[t * P:(t + 1) * P, :])
                    else:
                        nc.gpsimd.dma_start(out=x_tm[:], in_=xflat[t * P:(t + 1) * P, :])
                    pst = ps_tr.tile([P, P], ACT, tag="tr")
                    nc.tensor.transpose(pst[0:C, :], x_tm[:], ident[:])
                    nc.vector.tensor_copy(out=xt[0][0][:, t * P:(t + 1) * P],
                                          in_=pst[0:C, :])

            # ---- phase B: K/V projection for own half, write kvh ----
            if "B" not in _phases:
                continue
            kv_writes = []
            for t in range(nt):
                psK = ps.tile([P, C], F32, tag="pp")
                psV = ps.tile([P, C], F32, tag="pp")
                mm(psK[:], ones[0:1, 0:P], wsb[s]["bqkv"][0:1, C:2 * C], True, False)
                mm(psV[:], ones[0:1, 0:P], wsb[s]["bqkv"][0:1, 2 * C:3 * C], True, False)
                for i, (c0, c1) in enumerate(CH):
                    last = i == len(CH) - 1
                    lhs = xt[s][i][:, t * P:(t + 1) * P]
                    mm(psK[:], lhs, wsb[s]["wqkv"][i][:, C:2 * C], False, last)
                    mm(psV[:], lhs, wsb[s]["wqkv"][i][:, 2 * C:3 * C], False, last)
                kv_sb = sb.tile([P, R], ACT, tag="kv_sb")
                if R != 2 * C:
                    nc.vector.memset(kv_sb[:, 2 * C:R], 0.0)
                nc.vector.tensor_copy(out=kv_sb[:, 0:C], in_=psK[:])
                nc.vector.tensor_copy(out=kv_sb[:, C:2 * C], in_=psV[:])
                d = nc.sync.dma_start(out=kvh[s][t * P:(t + 1) * P, :], in_=kv_sb[:])
                kv_writes.append(d)

            # ---- KV exchange with twin core ----
            if cfg.collective:
                cc = nc.gpsimd.collective_compute(
                    kind="AllGather", op=ALU.bypass,
                    replica_groups=[[2 * i, 2 * i + 1] for i in range(cfg.n_pairs)],
                    ins=[kvh[s][:]], outs=[kvf[s][:]])
                for d in kv_writes:
                    tile.add_dep_helper(cc.ins, d.ins, sync=True,
                                        reason="kv allgather after kv writes")
                tile.add_dep_helper(cc.ins, ll.ins, sync=True,
                                    reason="library before collective")
            else:
                if os.environ.get("KM_NOJOIN"):
                    cc = None
                else:
                    cc = nc.gpsimd.engine_nop()
                    for d in kv_writes:
                        tile.add_dep_helper(cc.ins, d.ins, sync=True,
                                            reason="kv joiner after kv writes")
                    tile.add_dep_helper(cc.ins, ll.ins, sync=True,
                                        reason="library before gathers")

            # ---- phase C: attention per gather block ----
            if "C" not in _phases:
                continue
            scr_writes = []
            for blk in range(nblocks):
                # Q projection for the G tiles of this block
                q_sb = sb.tile([P, G * C], ACT, tag="q_sb")
                for g in range(G):
                    t = blk * G + g
                    psQ = ps.tile([P, C], F32, tag="pp")
                    mm(psQ[:], ones[0:1, 0:P], wsb[s]["bqkv"][0:1, 0:C], True, False)
                    for i, (c0, c1) in enumerate(CH):
                        mm(psQ[:], xt[s][i][:, t * P:(t + 1) * P],
                           wsb[s]["wqkv"][i][:, 0:C], False, i == len(CH) - 1)
                    nc.vector.tensor_copy(out=q_sb[:, g * C:(g + 1) * C], in_=psQ[:])

                # gather neighbor KV rows
                _clvl = int(os.environ.get("KM_CLVL", "9"))
                if _clvl < 2:
                    continue
                nidx = G * P * KN
                gk = gp.tile([P, G * KN * R], ACT, tag="gk")
                gi = nc.gpsimd.dma_gather(
                    out_ap=gk[:].rearrange("p (gj r) -> p gj r", r=R), in_ap=kvf[s][:],
                    idxs_ap=wsb[s]["idx"][:, blk * (nidx // 16):(blk + 1) * (nidx // 16)],
                    num_idxs=nidx, num_idxs_reg=nidx, elem_size=R,
                    single_packet=False)
                if cc is not None:
                    tile.add_dep_helper(gi.ins, cc.ins, sync=True,
                                        reason="gather after kv exchange")

                if _clvl < 3:
                    continue
                gk4 = gk[:].rearrange("p (g j r) -> p g j r", g=G, j=KN, r=R)
                k4 = gk4[:, :, :, 0:C]
                v4 = gk4[:, :, :, C:2 * C]
                q3 = q_sb[:].rearrange("p (g c) -> p g c", g=G, c=C)
                q4 = bass.AP(q3.tensor, q3.offset,
                             [list(q3.ap[0]), list(q3.ap[1]), [0, KN], list(q3.ap[2])])

                prod = gp.tile([P, G * KN * C], ACT, tag="prod")
                prod4 = prod[:].rearrange("p (g j c) -> p g j c", g=G, j=KN, c=C)
                nc.vector.tensor_tensor(out=prod4, in0=k4, in1=q4, op=ALU.mult)
                sc = sb.tile([P, G * KN], F32, tag="sc")
                sc3 = sc[:].rearrange("p (g j) -> p g j", g=G, j=KN)
                nc.vector.tensor_reduce(out=sc3, in_=prod4, axis=AX.X, op=ALU.add)
                if _clvl < 4:
                    continue
                # + relative bias (precomputed [128, 8], same for every tile)
                b3 = wsb[s]["b128"][:]
                b4 = bass.AP(b3.tensor, b3.offset,
                             [list(b3.ap[0]), [0, G], list(b3.ap[1])])
                scb = sb.tile([P, G * KN], F32, tag="scb")
                scb3 = scb[:].rearrange("p (g j) -> p g j", g=G, j=KN)
                nc.vector.tensor_tensor(out=scb3, in0=sc3, in1=b4, op=ALU.add)
                # softmax over the 8 neighbors (scores are O(1), skip max-sub)
                ex = sb.tile([P, G * KN], F32, tag="ex")
                nc.scalar.activation(out=ex[:], in_=scb[:], func=AF.Exp,
                                     bias=zero1[:])
                ex3 = ex[:].rearrange("p (g j) -> p g j", g=G, j=KN)
                den = sb.tile([P, G], F32, tag="den")
                nc.vector.tensor_reduce(out=den[:], in_=ex3, axis=AX.X, op=ALU.add)
                rec = sb.tile([P, G], F32, tag="rec")
                nc.vector.reciprocal(rec[:], den[:])
                r3 = rec[:]
                r4 = bass.AP(r3.tensor, r3.offset,
                             [list(r3.ap[0]), list(r3.ap[1]), [0, KN]])
                pr = sb.tile([P, G * KN], F32, tag="pr")
                pr3 = pr[:].rearrange("p (g j) -> p g j", g=G, j=KN)
                nc.vector.tensor_tensor(out=pr3, in0=ex3, in1=r4, op=ALU.mult)

                if _clvl < 5:
                    continue
                # PV: weighted neighbor values (ACT does the per-(g,j) scaling)
                pv = gp.tile([P, G * KN * C], ACT, tag="prod")
                for g in range(G):
                    for j in range(KN):
                        nc.scalar.mul(
                            out=pv[:, (g * KN + j) * C:(g * KN + j + 1) * C],
                            in_=gk[:, (g * KN + j) * R + C:(g * KN + j) * R + 2 * C],
                            mul=pr[:, g * KN + j:g * KN + j + 1])
                pv4 = pv[:].rearrange("p (g j c) -> p g j c", g=G, j=KN, c=C)
                o4 = sb.tile([P, G * 4 * C], ACT, tag="o4", bufs=2)
                o44 = o4[:].rearrange("p (g j c) -> p g j c", g=G, j=4, c=C)
                nc.vector.tensor_tensor(out=o44, in0=pv4[:, :, 0:4, :],
                                        in1=pv4[:, :, 4:8, :], op=ALU.add)
                o2 = sb.tile([P, G * 2 * C], ACT, tag="o2", bufs=2)
                o24 = o2[:].rearrange("p (g j c) -> p g j c", g=G, j=2, c=C)
                nc.vector.tensor_tensor(out=o24, in0=o44[:, :, 0:2, :],
                                        in1=o44[:, :, 2:4, :], op=ALU.add)
                ope = sb.tile([P, G * C], ACT, tag="ope", bufs=2)
                ope3 = ope[:].rearrange("p (g c) -> p g c", g=G, c=C)
                nc.vector.tensor_tensor(out=ope3, in0=o24[:, :, 0, :],
                                        in1=o24[:, :, 1, :], op=ALU.add)

                if _clvl < 6:
                    continue
                # output projection + skip write, per tile
                for g in range(G):
                    t = blk * G + g
                    oT = []
                    for i, (c0, c1) in enumerate(CH):
                        pst = ps_tr.tile([P, P], ACT, tag="tr")
                        nc.tensor.transpose(pst[0:c1 - c0, :],
                                            ope[:, g * C + c0:g * C + c1], ident[:])
                        ot = sb.tile([P, P], ACT, tag=f"oT{i}", bufs=2)
                        nc.vector.tensor_copy(out=ot[0:c1 - c0, :], in_=pst[0:c1 - c0, :])
                        oT.append(ot)
                    psO = ps.tile([P, C], F32, tag="pp")
                    mm(psO[:], ones[0:1, 0:P], wsb[s]["bo"][0:1, :], True, False)
                    for i, (c0, c1) in enumerate(CH):
                        mm(psO[:], oT[i][0:c1 - c0, :], wsb[s]["wo"][i][:],
                           False, i == len(CH) - 1)
                    sk = sb.tile([P, C], F32, tag="sk")
                    nc.vector.tensor_copy(out=sk[:], in_=psO[:])
                    nc.sync.dma_start(out=outs[s][t * P:(t + 1) * P, :], in_=sk[:])
                    d = nc.sync.dma_start(out=skip_scr[s][t * P:(t + 1) * P, :],
                                          in_=sk[:])
                    scr_writes.append(d)

            # ---- phase D: patch merge ----
            if "D" not in _phases:
                continue
            scr_join = nc.vector.engine_nop()
            for d in scr_writes:
                tile.add_dep_helper(scr_join.ins, d.ins, sync=True,
                                    reason="merge joiner after skip_scr writes")
            mtok = Nh // 4
            n_mt = (mtok + P - 1) // P
            CH4 = _chunks(4 * C)
            scr5 = skip_scr[s].rearrange("(h2 a w2 b) c -> h2 a w2 b c",
                                         a=2, b=2, w2=W // 2)
            for mt in range(n_mt):
                mtp = min(P, mtok - mt * P)
                rows_per_tile = mtp // (W // 2)
                mh0 = (mt * P) // (W // 2)
                mg = sb.tile([P, 4 * C], F32, tag="mg", bufs=2)
                for hs in range(2):
                    for ws in range(2):
                        src = scr5[mh0:mh0 + rows_per_tile, hs, :, ws, :]
                        dst = mg[0:mtp, (2 * hs + ws) * C:(2 * hs + ws + 1) * C]
                        d = nc.sync.dma_start(out=dst, in_=src)
                        tile.add_dep_helper(d.ins, scr_join.ins, sync=True,
                                            reason="merge read after skip_scr")
                red = sb.tile([P, 1], F32, tag="red")
                nc.vector.tensor_reduce(out=red[0:mtp, :], in_=mg[0:mtp, :],
                                        axis=AX.X, op=ALU.add)
                nm = sb.tile([P, 1], F32, tag="nm")
                nc.vector.tensor_scalar_mul(nm[0:mtp, :], red[0:mtp, :], -1.0 / (4 * C))
                sq = sb.tile([P, 4 * C], F32, tag="sq", bufs=2)
                ssq = sb.tile([P, 1], F32, tag="ssq")
                nc.scalar.activation(out=sq[0:mtp, :], in_=mg[0:mtp, :], func=AF.Square,
                                     bias=nm[0:mtp, :], scale=1.0,
                                     accum_out=ssq[0:mtp, :])
                std = sb.tile([P, 1], F32, tag="std")
                nc.scalar.activation(out=std[0:mtp, :], in_=ssq[0:mtp, :], func=AF.Sqrt,
                                     scale=1.0 / (4 * C), bias=eps1[0:mtp, :])
                rstd = sb.tile([P, 1], F32, tag="rstd")
                nc.vector.reciprocal(rstd[0:mtp, :], std[0:mtp, :])
                b2 = sb.tile([P, 1], F32, tag="b2")
                nc.vector.tensor_tensor(out=b2[0:mtp, :], in0=nm[0:mtp, :],
                                        in1=rstd[0:mtp, :], op=ALU.mult)
                nrm = sb.tile([P, 4 * C], ACT, tag="nrm", bufs=2)
                nc.scalar.activation(out=nrm[0:mtp, :], in_=mg[0:mtp, :],
                                     func=AF.Identity, bias=b2[0:mtp, :],
                                     scale=rstd[0:mtp, :])
                # transpose normalized tile into feature-major chunks
                nT = []
                for i, (c0, c1) in enumerate(CH4):
                    pst = ps_tr.tile([P, P], ACT, tag="tr")
                    nc.tensor.transpose(pst[0:c1 - c0, 0:mtp],
                                        nrm[0:mtp, c0:c1], ident[0:mtp, 0:mtp])
                    ntl = sb.tile([P, P], ACT, tag=f"nT{i}", bufs=1)
                    nc.vector.tensor_copy(out=ntl[0:c1 - c0, 0:mtp],
                                          in_=pst[0:c1 - c0, 0:mtp])
                    nT.append(ntl)
                if s < 2:
                    # feature-major output -> xT of next stage
                    for mi, (m0, m1) in enumerate(_chunks(2 * C)):
                        ml = m1 - m0
                        psM = ps.tile([P, P], F32, tag="pp")
                        mm(psM[0:ml, 0:mtp], wsb[s]["brp"][0:1, m0:m1],
                           ones[0:1, 0:mtp], True, False)
                        for i, (c0, c1) in enumerate(CH4):
                            mm(psM[0:ml, 0:mtp], wsb[s]["wrp"][i][:, m0:m1],
                               nT[i][0:c1 - c0, 0:mtp], False, i == len(CH4) - 1)
                        nc.vector.tensor_copy(
                            out=xt[s + 1][mi][:, mt * P:mt * P + mtp],
                            in_=psM[0:ml, 0:mtp])
                else:
                    # token-major final output
                    xo = sb.tile([P, 2 * C], F32, tag="xo")
                    half_cols = C  # 2C split into two <=512 column halves
                    for ch in range(2):
                        csl = slice(ch * C, (ch + 1) * C)
                        psF = ps.tile([P, C], F32, tag="pp")
                        mm(psF[0:mtp, :], ones[0:1, 0:mtp], wsb[s]["brp"][0:1, csl],
                           True, False)
                        for i, (c0, c1) in enumerate(CH4):
                            mm(psF[0:mtp, :], nT[i][0:c1 - c0, 0:mtp],
                               wsb[s]["wrp"][i][:, csl], False, i == len(CH4) - 1)
                        nc.vector.tensor_copy(out=xo[0:mtp, csl], in_=psF[0:mtp, :])
                    nc.sync.dma_start(out=xout[mt * P:mt * P + mtp, :],
                                      in_=xo[0:mtp, :])

        for pool_cm in (ps_tr_cm, ps_cm, gp_cm, sb_cm, wp_cm):
            pool_cm.__exit__(None, None, None)

    nc.compile()
    return nc


# --------------------------------------------------------- host wrapper -----
_CACHE = {}


def _np_act(a, act_dt):
    a = np.asarray(a, dtype=np.float32)
    if act_dt == BF16:
        return a.astype(ml_dtypes.bfloat16)
    return a


def _prep_idx(nbrs_half, G):
    """Reorder neighbor table for dma_gather: flat[(g*8+j)*128+p] = nbrs[g*128+p, j],
    then 16-partition wrap, then replicate to 128 partitions."""
    Nh = nbrs_half.shape[0]
    nt = Nh // P
    nb = nt // G
    blocks = []
    for b in range(nb):
        a = nbrs_half[b * G * P:(b + 1) * G * P].reshape(G, P, KN)
        a = a.transpose(0, 2, 1).reshape(-1)  # [(g j) p]
        blocks.append(a.reshape(-1, 16).T)  # wrap 16
    arr = np.concatenate(blocks, axis=1)
    return np.tile(arr, (8, 1)).astype(np.int16)


def _prep_core_inputs(cfg, x_np, nbrs, params):
    """Per-(batch,half) input maps. nbrs: list of 3 [N,8] arrays."""
    act = cfg.act_dt
    shared = {}
    for s, st in enumerate(cfg.stages):
        C = st["C"]
        p = params[f"s{s}"]
        wq = np.asarray(p["wq"], np.float32) / np.sqrt(C)
        bq = np.asarray(p["bq"], np.float32) / np.sqrt(C)
        wqkv = np.concatenate([wq, np.asarray(p["wk"], np.float32),
                               np.asarray(p["wv"], np.float32)], axis=1)
        bqkv = np.concatenate([bq, np.asarray(p["bk"], np.float32),
                               np.asarray(p["bv"], np.float32)])[None, :]
        g = np.asarray(p["g"], np.float32)
        be = np.asarray(p["be"], np.float32)
        wr = np.asarray(p["wr"], np.float32)
        wrp = wr * g[:, None]
        brp = (be @ wr)[None, :]
        rb = np.asarray(p["rb"], np.float32)
        b128 = rb[np.arange(P) % KN]
        shared[f"wqkv{s}"] = _np_act(wqkv, act)
        shared[f"bqkv{s}"] = _np_act(bqkv, act)
        shared[f"wo{s}"] = _np_act(np.asarray(p["wo"], np.float32), act)
        shared[f"bo{s}"] = _np_act(np.asarray(p["bo"], np.float32)[None, :], act)
        shared[f"wrp{s}"] = _np_act(wrp, act)
        shared[f"brp{s}"] = _np_act(brp, act)
        shared[f"b128_{s}"] = b128.astype(np.float32)
    shared["ident"] = _np_act(np.eye(P, dtype=np.float32), act)

    in_maps = []
    hrows = cfg.stages[0]["H"] // cfg.split
    for b in range(cfg.n_pairs):
        for half in range(cfg.split):
            m = dict(shared)
            m["x"] = np.ascontiguousarray(
                x_np[b, half * hrows:(half + 1) * hrows], dtype=np.float32)
            for s, st in enumerate(cfg.stages):
                Nh = st["Nh"]
                nb_half = nbrs[s][half * Nh:(half + 1) * Nh]
                m[f"idx{s}"] = _prep_idx(nb_half, st["G"])
            in_maps.append(m)
    return in_maps


def run_encoder(cfg, x_np, nbrs, params, trace=False):
    key = cfg.key()
    if key not in _CACHE:
        _CACHE[key] = build_encoder(cfg)
    nc = _CACHE[key]
    in_maps = _prep_core_inputs(cfg, x_np, nbrs, params)
    res = run_bass_kernel_spmd(nc, in_maps,
                               core_ids=list(range(cfg.split * cfg.n_pairs)),
                               trace=trace)
    B = cfg.n_pairs
    sts = cfg.stages
    skips = []
    for s, st in enumerate(sts):
        N, Nh, C = st["N"], st["Nh"], st["C"]
        out = np.empty((B, N, C), np.float32)
        for b in range(B):
            for half in range(cfg.split):
                out[b, half * Nh:(half + 1) * Nh] = \
                    res.results[cfg.split * b + half][f"skip{s}"]
        skips.append(out)
    H3, W3 = sts[2]["H"] // 2, sts[2]["W"] // 2
    Cf = 2 * sts[2]["C"]
    xfin = np.empty((B, H3, W3, Cf), np.float32)
    hr3 = H3 // cfg.split
    for b in range(B):
        for half in range(cfg.split):
            part = res.results[cfg.split * b + half]["xout2"]
            xfin[b, half * hr3:(half + 1) * hr3] = part.reshape(hr3, W3, Cf)
    return (xfin, skips[0], skips[1], skips[2]), res


def kernel(x, neighbors1, neighbors2, neighbors3, params):
    x = np.asarray(x, np.float32)
    nbrs = [np.asarray(n, np.int32) for n in (neighbors1, neighbors2, neighbors3)]
    cfg = Cfg(H0=x.shape[1], W0=x.shape[2], C0=x.shape[3], n_pairs=x.shape[0])
    out, _ = run_encoder(cfg, x, nbrs, params)
    return out
